# revision 22
# baseline (speedup 1.0000x reference)
"""MultiHeadLinearAttention Trainium2 kernel (8 NeuronCores, SPMD).

Sharding: core c handles batch b = c//2, head-group g = c%2 (4 of 8 heads,
i.e. feature slice F = [256g, 256g+256) of the 512 projection features).
Each core computes k/v/q projections restricted to its head-group, the
per-head linear-attention state over the full 8192-token sequence, and a
partial output  attn_F @ out_w[:, F].T.  The host sums the two partials per
batch and adds out_b.  No cross-core collectives are needed.

Math per head h (matches the fp32 jax reference):
  proj(x)  = silu(x@w1.T + b1) * (x@w2.T + b2)
  phi(x)   = elu(x) + 1 = max(x+1, exp(min(x, 0)))
  kv[d,e]  = sum_s phi_k[s,d] v[s,e]        (64x64 per head)
  ksum[d]  = sum_s phi_k[s,d]
  attn[s,e]= (sum_d phi_q[s,d] kv[d,e]) / (sum_d phi_q[s,d] ksum[d])
  out      = attn @ out_w.T + out_b
(The reference's +1e-6 in the denominator is negligible: denominators are
O(1e5) here.)

Perf notes (from NTFF traces):
  - matmul operands in fp16 (1 cyc/row at any free-dim; fp32r is 4 cyc/row
    below free-dim 256, which hit the state matmuls). Inputs are fp16 in
    DRAM -> half the HBM read traffic.
  - Activation table reloads (~1.5us each) dominate the scalar engine if
    Silu and Exp alternate (no act-func set holds both). All Silu work of
    a group of chunks is issued before all Exp work -> ~2 reloads/group.
  - Engines reading PSUM keep fp32 I/O (fp16 out on ACT/DVE measured
    slower); only the four matmul feeds (phik, vproj, phm, att) are fp16.
  - ksum is folded into the kv state matmul as 2 constant-1.0 columns of
    vproj (free dim 130), halving phase-1 state matmul+ldweights count.
  - Inputs are host-tiled to [128, chunk, ko, tok] so each chunk loads
    with ONE 4KB-per-partition-line DMA (descriptor-bound otherwise).
  - Reciprocal via the custom-DVE reciprocal_approx_fast (~5x faster,
    ~18 bits; denominators are O(1e4..1e5), well inside its safe range).
"""
import sys
sys.path.insert(0, '/opt/trn_rl_repo')

import numpy as np
import concourse.bass as bass
import concourse.mybir as mybir
import concourse.tile as tile
from concourse.bass import ts, ds
from concourse.bass_utils import run_bass_kernel_spmd

F32 = mybir.dt.float32
F32R = mybir.dt.float32r
F16 = mybir.dt.float16
AF = mybir.ActivationFunctionType
OP = mybir.AluOpType

B, S, D = 4, 8192, 512
NH, DK = 8, 64
FG = 256            # features per head-group (4 heads)
P = 128
CHUNK = 512         # tokens per streamed chunk
NCHUNK = S // CHUNK         # 16
SUBT = CHUNK // P           # 4 subtiles of 128 tokens per chunk
GRP1 = 8            # chunks per phase-1 act-batch group
GRP2 = 4            # chunks per phase-2 act-batch group
STW = 130           # state width per pair: 128 kv cols + 2 ksum cols


def _split_waits(nc, limit=1):
    """walrus here rejects >1 embedded sync-wait per instruction; move extras
    onto same-engine NoOps immediately before (program order preserves
    semantics)."""
    uid = 0
    for f in nc.m.functions:
        for blk in f.blocks:
            new = []
            for ins in blk.instructions:
                si = ins.sync_info
                if si is not None and si.on_wait is not None and len(si.on_wait) > limit:
                    waits = list(si.on_wait)
                    head, keep = waits[:-limit], waits[-limit:]
                    for w in head:
                        nop = mybir.InstNoOp(
                            name=f"wsplit_{uid}", ins=[], outs=[],
                            sync_info=mybir.SyncInfo(on_wait=[w], on_update=[]))
                        uid += 1
                        nop.engine = ins.engine
                        new.append(nop)
                    ins.sync_info = mybir.SyncInfo(
                        on_wait=keep, on_update=list(si.on_update or []))
                new.append(ins)
            blk.instructions = new


def build_nc(repeats=1):
    nc = bass.Bass()

    # --- DRAM I/O (per-core data supplied via in_maps) ---
    # x*_t: [128, chunk, ko, tok] so one chunk = one DMA, 4KB lines.
    xk_t = nc.dram_tensor("xk_t", [P, NCHUNK, 4, CHUNK], F16, kind="ExternalInput")
    xv_t = nc.dram_tensor("xv_t", [P, NCHUNK, 4, CHUNK], F16, kind="ExternalInput")
    xq_t = nc.dram_tensor("xq_t", [P, NCHUNK, 4, CHUNK], F16, kind="ExternalInput")
    wk12T = nc.dram_tensor("wk12T", [D, 2 * FG], F16, kind="ExternalInput")
    wv12T = nc.dram_tensor("wv12T", [D, 2 * FG], F16, kind="ExternalInput")
    wq1T = nc.dram_tensor("wq1T", [D, FG], F16, kind="ExternalInput")
    wq2T = nc.dram_tensor("wq2T", [D, FG], F16, kind="ExternalInput")
    bk12p = nc.dram_tensor("bk12p", [P, 2 * FG], F16, kind="ExternalInput")
    bv12p = nc.dram_tensor("bv12p", [P, 2 * FG], F16, kind="ExternalInput")
    bq1 = nc.dram_tensor("bq1", [P, 2], F32, kind="ExternalInput")
    bq2 = nc.dram_tensor("bq2", [P, 2], F32, kind="ExternalInput")
    woT = nc.dram_tensor("woT", [FG, D], F16, kind="ExternalInput")
    e0 = nc.dram_tensor("e0", [P, P], F16, kind="ExternalInput")      # row0=1
    sel = nc.dram_tensor("sel", [P, P], F32R, kind="ExternalInput")
    rcp_init = nc.dram_tensor("rcp_init", [P, 4 * CHUNK], F32R, kind="ExternalInput")
    bdz = nc.dram_tensor("bdz", [P, 2 * P], F16, kind="ExternalInput")
    dkz = nc.dram_tensor("dkz", [P, 4], F16, kind="ExternalInput")
    out = nc.dram_tensor("out", [S, D], F32, kind="ExternalOutput")

    wk12T_r = wk12T.rearrange("(ko p) o -> p ko o", p=P)   # [128, 4, 512]
    wv12T_r = wv12T.rearrange("(ko p) o -> p ko o", p=P)
    wq1T_r = wq1T.rearrange("(ko p) o -> p ko o", p=P)     # [128, 4, 256]
    wq2T_r = wq2T.rearrange("(ko p) o -> p ko o", p=P)
    woT_r = woT.rearrange("(ko p) o -> p ko o", p=P)       # [128, 2, 512]
    out_r = out.rearrange("(n p) f -> n p f", p=P)         # [64, 128, 512]

    with tile.TileContext(nc) as tc:
        with tc.tile_pool(name="const", bufs=1) as cpool:
            # Resident weights / constants
            wk_sb = cpool.tile([P, 4, 2 * FG], F16)
            wv_sb = cpool.tile([P, 4, 2 * FG], F16)
            wq1_sb = cpool.tile([P, 4, FG], F16)
            wq2_sb = cpool.tile([P, 4, FG], F16)
            wo_sb = cpool.tile([P, 2, D], F16)
            bk_sb = cpool.tile([P, 2 * FG], F16)
            bv_sb = cpool.tile([P, 2 * FG], F16)
            bq1_sb = cpool.tile([P, 2], F32)
            bq2_sb = cpool.tile([P, 2], F32)
            e0_sb = cpool.tile([P, P], F16)
            sel_sb = cpool.tile([P, P], F32R)
            nc.sync.dma_start(e0_sb[:], e0[:])
            nc.sync.dma_start(bk_sb[:], bk12p[:])
            nc.sync.dma_start(bv_sb[:], bv12p[:])

            # Per-head-pair numerator/denominator lhsT built at phase boundary
            bd_sb = cpool.tile([P, 2, P], F16)      # blockdiag kv per pair
            dk_sb = cpool.tile([P, 2, 2], F16)      # ksum columns per pair

            # reciprocal staging (double-buffered); denominators land in rows
            # 0:2 (pair0) and 32:34 (pair1); other rows stay 1.0 so the
            # sel-matmul reads defined data everywhere it is nonzero.
            rcp_sb = cpool.tile([P, 2, 2, CHUNK], F32R)

            for _rep in range(repeats):
              # ---------------- Phase 1: k/v projections + state ----------------
              ctx_iop2 = tc.tile_pool(name="p2_io", bufs=3)
              iop2 = ctx_iop2.__enter__()
              with tc.tile_pool(name="p1_io", bufs=3) as iop, \
                   tc.tile_pool(name="p1_sil", bufs=4) as silp, \
                   tc.tile_pool(name="p1_kp", bufs=4 * GRP1 + 2) as kpp, \
                   tc.tile_pool(name="p1_ex", bufs=4) as exp_, \
                   tc.tile_pool(name="p1_ps", bufs=3, space="PSUM") as psp, \
                   tc.tile_pool(name="p1_st", bufs=1, space="PSUM") as stp:

                  state_ps = stp.tile([P, 2 * STW], F32)  # [kv|ksum2] per pair

                  for g in range(NCHUNK // GRP1):
                      kprojs, vprojs, mnks = [], [], []
                      for ci in range(GRP1):
                          c = g * GRP1 + ci
                          kT_c = iop.tile([P, 4, CHUNK], F16, tag="kT")
                          vT_c = iop.tile([P, 4, CHUNK], F16, tag="vT")
                          nc.sync.dma_start(kT_c[:], xk_t[:, c])
                          if c == 0:
                              for ki in range(4):
                                  nc.sync.dma_start(wk_sb[:, ki, :], wk12T_r[:, ki, :])
                          nc.sync.dma_start(vT_c[:], xv_t[:, c])
                          if c == 0:
                              for ki in range(4):
                                  nc.sync.dma_start(wv_sb[:, ki, :], wv12T_r[:, ki, :])
                          for s in range(SUBT):
                              tok = ds(s * P, P)
                              psk = psp.tile([P, 2 * FG], F32, tag="proj")
                              psv = psp.tile([P, 2 * FG], F32, tag="proj")
                              nc.tensor.matmul(psk[:], e0_sb[:], bk_sb[:], start=True, stop=False)
                              for ki in range(4):
                                  nc.tensor.matmul(psk[:], kT_c[:, ki, tok], wk_sb[:, ki, :],
                                                   start=False, stop=(ki == 3))
                              nc.tensor.matmul(psv[:], e0_sb[:], bv_sb[:], start=True, stop=False)
                              for ki in range(4):
                                  nc.tensor.matmul(psv[:], vT_c[:, ki, tok], wv_sb[:, ki, :],
                                                   start=False, stop=(ki == 3))
                              # silu halves (Silu-run on scalar engine)
                              silk = silp.tile([P, FG], F32, tag="silk")
                              nc.scalar.activation(silk[:], psk[:, :FG], AF.Silu)
                              silv = silp.tile([P, FG], F32, tag="silv")
                              nc.scalar.activation(silv[:], psv[:, :FG], AF.Silu)
                              kproj = kpp.tile([P, FG], F32, tag="kproj")
                              nc.vector.tensor_tensor(kproj[:], psk[:, FG:], silk[:], OP.mult)
                              vproj = kpp.tile([P, 2, STW], F16, tag="vproj")
                              with nc.allow_low_precision(reason='fp16 matmul feed'):
                                  nc.vector.tensor_tensor(
                                      vproj[:, :, 0:P],
                                      psv[:, FG:].rearrange("p (g m) -> p g m", m=P),
                                      silv[:].rearrange("p (g m) -> p g m", m=P),
                                      OP.mult)
                              nc.gpsimd.memset(vproj[:, :, P:STW], 1.0)
                              mnk = kpp.tile([P, FG], F32, tag="mnk")
                              nc.gpsimd.tensor_scalar_min(mnk[:], kproj[:], 0.0)
                              kprojs.append(kproj)
                              vprojs.append(vproj)
                              mnks.append(mnk)
                      # group tail: Exp-run + phi + state accumulation
                      for i in range(4 * GRP1):
                          gi = g * 4 * GRP1 + i
                          first = (gi == 0)
                          last = (gi == S // P - 1)
                          exk = exp_.tile([P, FG], F32, tag="exk")
                          nc.scalar.activation(exk[:], mnks[i][:], AF.Exp)
                          phik = exp_.tile([P, FG], F16, tag="phik")
                          with nc.allow_low_precision(reason='fp16 matmul feed'):
                              nc.vector.scalar_tensor_tensor(
                                  phik[:], kprojs[i][:], 1.0, exk[:], OP.add, OP.max)
                          # State: one matmul per pair covers kv (128 cols) and
                          # ksum (2 constant-1.0 cols of vproj). ONE PSUM bank
                          # holds both pairs' regions: start=True clears
                          # has_written for the WHOLE bank, so only the very
                          # first state matmul may use it; the other region's
                          # first matmul overwrites (bits cleared) and sets its
                          # own bits, after which everything accumulates.
                          for p in range(2):
                              nc.tensor.matmul(state_ps[:, ds(p * STW, STW)],
                                               phik[:, ts(p, P)], vprojs[i][:, p, :],
                                               start=(first and p == 0), stop=last,
                                               skip_group_check=True)

                  # --- phase boundary: build bd (blockdiag kv) and dk (ksum cols)
                  nc.sync.dma_start(bd_sb[:], bdz.rearrange("p (g m) -> p g m", m=P))
                  nc.sync.dma_start(dk_sb[:], dkz.rearrange("p (g m) -> p g m", m=2))
                  with nc.allow_low_precision(reason='fp16 state for PE matmul'):
                      for p in range(2):
                          nc.vector.tensor_copy(bd_sb[0:64, p, 0:64],
                                                state_ps[0:64, ds(p * STW, 64)])
                          nc.vector.tensor_copy(bd_sb[64:P, p, 64:P],
                                                state_ps[64:P, ds(p * STW + 64, 64)])
                          nc.vector.tensor_copy(dk_sb[0:64, p, 0:1],
                                                state_ps[0:64, ds(p * STW + P, 1)])
                          nc.vector.tensor_copy(dk_sb[64:P, p, 1:2],
                                                state_ps[64:P, ds(p * STW + P + 1, 1)])

              # phase-2 weights load late so phase-1's first tiles win the DMA queue
              nc.sync.dma_start(wq1_sb[:], wq1T_r[:])
              nc.sync.dma_start(wq2_sb[:], wq2T_r[:])
              nc.sync.dma_start(wo_sb[:], woT_r[:])
              nc.sync.dma_start(bq1_sb[:], bq1[:])
              nc.sync.dma_start(bq2_sb[:], bq2[:])
              nc.sync.dma_start(sel_sb[:], sel[:])
              nc.sync.dma_start(rcp_sb[:], rcp_init.rearrange("p (a b t) -> p a b t", a=2, b=2))

              # ---------------- Phase 2: q projections + attention + out -------
              with tc.tile_pool(name="p2_sb", bufs=4) as sbp2, \
                   tc.tile_pool(name="p2_qp", bufs=2 * GRP2 + 2) as qpp, \
                   tc.tile_pool(name="p2_ps", bufs=3, space="PSUM") as psp2, \
                   tc.tile_pool(name="p2_ps_big", bufs=3, space="PSUM") as psb2, \
                   tc.tile_pool(name="p2_ps_dn", bufs=2, space="PSUM") as psd2:

                  for g in range(NCHUNK // GRP2):
                      saved = []
                      for ci in range(GRP2):
                          c = g * GRP2 + ci
                          qT_c = iop2.tile([P, 4, CHUNK], F16, tag="qT")
                          nc.sync.dma_start(qT_c[:], xq_t[:, c])
                          qps, mnqs = [], []
                          for m in range(2):
                              ps1 = psp2.tile([P, CHUNK], F32, tag="qproj")
                              ps2 = psp2.tile([P, CHUNK], F32, tag="qproj")
                              for ki in range(4):
                                  nc.tensor.matmul(ps1[:], wq1_sb[:, ki, ts(m, P)],
                                                   qT_c[:, ki, :], start=(ki == 0), stop=(ki == 3))
                              for ki in range(4):
                                  nc.tensor.matmul(ps2[:], wq2_sb[:, ki, ts(m, P)],
                                                   qT_c[:, ki, :], start=(ki == 0), stop=(ki == 3))
                              sil = sbp2.tile([P, CHUNK], F32, tag="sil")
                              nc.scalar.activation(sil[:], ps1[:], AF.Silu,
                                                   bias=bq1_sb[:, ds(m, 1)], scale=1.0)
                              qp = qpp.tile([P, CHUNK], F32, tag="qp")
                              nc.vector.scalar_tensor_tensor(
                                  qp[:], ps2[:], bq2_sb[:, ds(m, 1)], sil[:], OP.add, OP.mult)
                              mnq = qpp.tile([P, CHUNK], F32, tag="mnq")
                              nc.gpsimd.tensor_scalar_min(mnq[:], qp[:], 0.0)
                              qps.append(qp)
                              mnqs.append(mnq)
                          saved.append((c, qps, mnqs))
                      # group tail: Exp-run + attention + output
                      for (c, qps, mnqs) in saved:
                          phiq = []
                          for m in range(2):
                              exq = sbp2.tile([P, CHUNK], F32, tag="exq")
                              nc.scalar.activation(exq[:], mnqs[m][:], AF.Exp)
                              phm = sbp2.tile([P, CHUNK], F16, tag="phiq")
                              with nc.allow_low_precision(reason='fp16 matmul feed'):
                                  nc.vector.scalar_tensor_tensor(
                                      phm[:], qps[m][:], 1.0, exq[:], OP.add, OP.max)
                              phiq.append(phm)

                          # denominators: [2,CHUNK] per pair (MM dst must start
                          # at partition 0, so one PSUM tile per pair)
                          dn0 = psd2.tile([2, CHUNK], F32, tag="dn")
                          dn1 = psd2.tile([2, CHUNK], F32, tag="dn")
                          nc.tensor.matmul(dn0[:], dk_sb[:, 0, :], phiq[0][:],
                                           start=True, stop=True)
                          nc.tensor.matmul(dn1[:], dk_sb[:, 1, :], phiq[1][:],
                                           start=True, stop=True)
                          par = c % 2
                          # reciprocal_approx_fast (fp32-only) into a scratch,
                          # then a scalar Copy (table-neutral) rounds to f32r
                          # for the PE broadcast matmul.
                          for pp, dnp in ((0, dn0), (1, dn1)):
                              rsc = sbp2.tile([2, CHUNK], F32, tag="rsc")
                              nc.vector.reciprocal_approx_fast(rsc[:], dnp[:])
                              with nc.allow_low_precision(reason='f32r PE feed'):
                                  nc.scalar.copy(rcp_sb[0:2, par, pp, :], rsc[:])

                          outp = []
                          for p in range(2):
                              # numerator
                              nump = psb2.tile([P, CHUNK], F32, tag="mm")
                              nc.tensor.matmul(nump[:], bd_sb[:, p, :], phiq[p][:],
                                               start=True, stop=True)
                              # reciprocal rows broadcast to the pair's 128 partitions
                              rb = psb2.tile([P, CHUNK], F32, tag="mm")
                              nc.tensor.matmul(rb[:], sel_sb[:],
                                               rcp_sb[:, par, p, :],
                                               start=True, stop=True)
                              rbs = sbp2.tile([P, CHUNK], F32, tag="rbs")
                              nc.scalar.copy(rbs[:], rb[:])
                              att = sbp2.tile([P, CHUNK], F16, tag="att")
                              with nc.allow_low_precision(reason='fp16 matmul feed'):
                                  nc.vector.tensor_tensor(att[:], nump[:], rbs[:], OP.mult)
                              outp.append(att)

                          for s in range(SUBT):
                              po = psb2.tile([P, D], F32, tag="mm")
                              nc.tensor.matmul(po[:], outp[0][:, ts(s, P)], wo_sb[:, 0, :],
                                               start=True, stop=False)
                              nc.tensor.matmul(po[:], outp[1][:, ts(s, P)], wo_sb[:, 1, :],
                                               start=False, stop=True)
                              ob = sbp2.tile([P, D], F32, tag="ob")
                              if s % 2 == 0:
                                  nc.scalar.copy(ob[:], po[:])
                              else:
                                  nc.vector.tensor_copy(ob[:], po[:])
                              nc.sync.dma_start(out_r[c * SUBT + s], ob[:])

              ctx_iop2.__exit__(None, None, None)

    from concourse.library_overlay import lower_extended_insts
    lower_extended_insts(nc)
    _split_waits(nc)
    return nc


_NC_CACHE = None


def _get_nc():
    global _NC_CACHE
    if _NC_CACHE is None:
        _NC_CACHE = build_nc()
    return _NC_CACHE


def _prep_in_maps(inputs):
    return _build_in_maps(
        inputs["query"], inputs["key"], inputs["value"],
        inputs["q_w1"], inputs["q_w2"], inputs["k_w1"], inputs["k_w2"],
        inputs["v_w1"], inputs["v_w2"], inputs["out_w"],
        inputs["q_b1"], inputs["q_b2"], inputs["k_b1"], inputs["k_b2"],
        inputs["v_b1"], inputs["v_b2"])


def _tile_x(xb):
    """[S, D] -> [128, NCHUNK, 4, CHUNK] fp16 per-core input layout."""
    a = np.asarray(xb, np.float32).T.astype(np.float16)   # [D, S]
    return np.ascontiguousarray(
        a.reshape(4, P, NCHUNK, CHUNK).transpose(1, 2, 0, 3))


def _build_in_maps(query, key, value,
                   q_w1, q_w2, k_w1, k_w2, v_w1, v_w2, out_w,
                   q_b1, q_b2, k_b1, k_b2, v_b1, v_b2):
    e0 = np.zeros((P, P), np.float16); e0[0, :] = 1.0
    # sel[k, m]: reciprocal row j (j=0,1) -> partitions 64j..64j+63
    sel = np.zeros((P, P), np.float32)
    sel[0, 0:64] = 1.0
    sel[1, 64:128] = 1.0
    rcp_init = np.ones((P, 4 * CHUNK), np.float32)
    bdz = np.zeros((P, 2 * P), np.float16)
    dkz = np.zeros((P, 4), np.float16)

    xk = [_tile_x(np.asarray(key)[b]) for b in range(B)]
    xv = [_tile_x(np.asarray(value)[b]) for b in range(B)]
    xq = [_tile_x(np.asarray(query)[b]) for b in range(B)]

    in_maps = []
    for c in range(8):
        b, g = c // 2, c % 2
        Fs = slice(FG * g, FG * (g + 1))
        bk12p = np.zeros((P, 2 * FG), np.float16)
        bk12p[0] = np.concatenate([np.asarray(k_b1)[Fs], np.asarray(k_b2)[Fs]])
        bv12p = np.zeros((P, 2 * FG), np.float16)
        bv12p[0] = np.concatenate([np.asarray(v_b1)[Fs], np.asarray(v_b2)[Fs]])
        in_maps.append({
            "xk_t": xk[b],
            "xv_t": xv[b],
            "xq_t": xq[b],
            "wk12T": np.ascontiguousarray(np.concatenate(
                [np.asarray(k_w1)[Fs].T, np.asarray(k_w2)[Fs].T], axis=1)).astype(np.float16),
            "wv12T": np.ascontiguousarray(np.concatenate(
                [np.asarray(v_w1)[Fs].T, np.asarray(v_w2)[Fs].T], axis=1)).astype(np.float16),
            "wq1T": np.ascontiguousarray(np.asarray(q_w1)[Fs].T).astype(np.float16),
            "wq2T": np.ascontiguousarray(np.asarray(q_w2)[Fs].T).astype(np.float16),
            "bk12p": bk12p,
            "bv12p": bv12p,
            "bq1": np.ascontiguousarray(np.asarray(q_b1)[Fs].reshape(2, P).T.astype(np.float32)),
            "bq2": np.ascontiguousarray(np.asarray(q_b2)[Fs].reshape(2, P).T.astype(np.float32)),
            "woT": np.ascontiguousarray(np.asarray(out_w)[:, Fs].T).astype(np.float16),
            "e0": e0, "sel": sel, "bdz": bdz, "dkz": dkz,
            "rcp_init": rcp_init,
        })
    return in_maps


def kernel(query, key, value,
           q_w1, q_w2, k_w1, k_w2, v_w1, v_w2, out_w,
           q_b1, q_b2, k_b1, k_b2, v_b1, v_b2, out_b):
    in_maps = _build_in_maps(query, key, value,
                             q_w1, q_w2, k_w1, k_w2, v_w1, v_w2, out_w,
                             q_b1, q_b2, k_b1, k_b2, v_b1, v_b2)
    nc = _get_nc()
    res = run_bass_kernel_spmd(nc, in_maps, core_ids=list(range(8)))
    ob = np.asarray(out_b, dtype=np.float32)
    out = np.empty((B, S, D), np.float32)
    for b in range(B):
        out[b] = res.results[2 * b]["out"] + res.results[2 * b + 1]["out"] + ob
    return out


# revision 28
# speedup vs baseline: 1.5841x; 1.5841x over previous
"""MultiHeadLinearAttention Trainium2 kernel (8 NeuronCores, SPMD).

Sharding: core c handles batch b = c//2, head-group g = c%2 (4 of 8 heads,
i.e. feature slice F = [256g, 256g+256) of the 512 projection features).
Each core computes k/v/q projections restricted to its head-group, the
per-head linear-attention state over the full 8192-token sequence, and a
partial output  attn_F @ out_w[:, F].T.  The host sums the two partials per
batch and adds out_b.  No cross-core collectives are needed.

Math per head h (matches the fp32 jax reference):
  proj(x)  = silu(x@w1.T + b1) * (x@w2.T + b2)
  phi(x)   = elu(x) + 1 = max(x+1, exp(min(x, 0)))
  kv[d,e]  = sum_s phi_k[s,d] v[s,e]        (64x64 per head)
  ksum[d]  = sum_s phi_k[s,d]
  attn[s,e]= (sum_d phi_q[s,d] kv[d,e]) / (sum_d phi_q[s,d] ksum[d])
  out      = attn @ out_w.T + out_b
(The reference's +1e-6 in the denominator is negligible: denominators are
O(1e5) here.)

Perf notes (from NTFF traces):
  - matmul operands in fp16 (1 cyc/row at any free-dim; fp32r is 4 cyc/row
    below free-dim 256, which hit the state matmuls). Inputs are fp16 in
    DRAM -> half the HBM read traffic.
  - Activation table reloads (~1.5us each) dominate the scalar engine if
    Silu and Exp alternate (no act-func set holds both). All Silu work of
    a group of chunks is issued before all Exp work -> ~2 reloads/group.
  - Engines reading PSUM keep fp32 I/O (fp16 out on ACT/DVE measured
    slower); only the four matmul feeds (phik, vproj, phm, att) are fp16.
  - ksum is folded into the kv state matmul as 2 constant-1.0 columns of
    vproj (free dim 130), halving phase-1 state matmul+ldweights count.
  - Inputs are host-tiled to [128, chunk, ko, tok] so each chunk loads
    with ONE 4KB-per-partition-line DMA (descriptor-bound otherwise).
  - Reciprocal via the custom-DVE reciprocal_approx_fast (~5x faster,
    ~18 bits; denominators are O(1e4..1e5), well inside its safe range).
"""
import sys
sys.path.insert(0, '/opt/trn_rl_repo')

import numpy as np
import concourse.bass as bass
import concourse.mybir as mybir
import concourse.tile as tile
from concourse.bass import ts, ds
from concourse.bass_utils import run_bass_kernel_spmd

F32 = mybir.dt.float32
F32R = mybir.dt.float32r
F16 = mybir.dt.float16
AF = mybir.ActivationFunctionType
OP = mybir.AluOpType

B, S, D = 4, 8192, 512
NH, DK = 8, 64
FG = 256            # features per head-group (4 heads)
P = 128
CHUNK = 512         # tokens per streamed chunk
NCHUNK = S // CHUNK         # 16
SUBT = CHUNK // P           # 4 subtiles of 128 tokens per chunk
GRP1 = 8            # chunks per phase-1 act-batch group
GRP2 = 4            # chunks per phase-2 act-batch group
STW = 130           # state width per pair: 128 kv cols + 2 ksum cols


def _split_waits(nc, limit=1):
    """walrus here rejects >1 embedded sync-wait per instruction; move extras
    onto same-engine NoOps immediately before (program order preserves
    semantics)."""
    uid = 0
    for f in nc.m.functions:
        for blk in f.blocks:
            new = []
            for ins in blk.instructions:
                si = ins.sync_info
                if si is not None and si.on_wait is not None and len(si.on_wait) > limit:
                    waits = list(si.on_wait)
                    head, keep = waits[:-limit], waits[-limit:]
                    for w in head:
                        nop = mybir.InstNoOp(
                            name=f"wsplit_{uid}", ins=[], outs=[],
                            sync_info=mybir.SyncInfo(on_wait=[w], on_update=[]))
                        uid += 1
                        nop.engine = ins.engine
                        new.append(nop)
                    ins.sync_info = mybir.SyncInfo(
                        on_wait=keep, on_update=list(si.on_update or []))
                new.append(ins)
            blk.instructions = new


def build_nc(repeats=1):
    nc = bass.Bass()

    # --- DRAM I/O (per-core data supplied via in_maps) ---
    # x*_t: [128, chunk, ko, tok] so one chunk = one DMA, 4KB lines.
    xk_t = nc.dram_tensor("xk_t", [P, NCHUNK, 4, CHUNK], F16, kind="ExternalInput")
    xv_t = nc.dram_tensor("xv_t", [P, NCHUNK, 4, CHUNK], F16, kind="ExternalInput")
    xq_t = nc.dram_tensor("xq_t", [P, NCHUNK, 4, CHUNK], F16, kind="ExternalInput")
    wk12T = nc.dram_tensor("wk12T", [D, 2 * FG], F16, kind="ExternalInput")
    wv12T = nc.dram_tensor("wv12T", [D, 2 * FG], F16, kind="ExternalInput")
    wq1T = nc.dram_tensor("wq1T", [D, FG], F16, kind="ExternalInput")
    wq2T = nc.dram_tensor("wq2T", [D, FG], F16, kind="ExternalInput")
    bk12p = nc.dram_tensor("bk12p", [P, 2 * FG], F16, kind="ExternalInput")
    bv12p = nc.dram_tensor("bv12p", [P, 2 * FG], F16, kind="ExternalInput")
    bq1 = nc.dram_tensor("bq1", [P, 2], F32, kind="ExternalInput")
    bq2 = nc.dram_tensor("bq2", [P, 2], F32, kind="ExternalInput")
    woT = nc.dram_tensor("woT", [FG, D], F16, kind="ExternalInput")
    e0 = nc.dram_tensor("e0", [P, P], F16, kind="ExternalInput")      # row0=1
    sel = nc.dram_tensor("sel", [P, P], F32R, kind="ExternalInput")
    rcp_init = nc.dram_tensor("rcp_init", [P, 4 * CHUNK], F32R, kind="ExternalInput")
    bdz = nc.dram_tensor("bdz", [P, 2 * P], F16, kind="ExternalInput")
    dkz = nc.dram_tensor("dkz", [P, 4], F16, kind="ExternalInput")
    out = nc.dram_tensor("out", [S, D], F32, kind="ExternalOutput")

    wk12T_r = wk12T.rearrange("(ko p) o -> p ko o", p=P)   # [128, 4, 512]
    wv12T_r = wv12T.rearrange("(ko p) o -> p ko o", p=P)
    wq1T_r = wq1T.rearrange("(ko p) o -> p ko o", p=P)     # [128, 4, 256]
    wq2T_r = wq2T.rearrange("(ko p) o -> p ko o", p=P)
    woT_r = woT.rearrange("(ko p) o -> p ko o", p=P)       # [128, 2, 512]
    out_r = out.rearrange("(n p) f -> n p f", p=P)         # [64, 128, 512]

    with tile.TileContext(nc) as tc:
        with tc.tile_pool(name="const", bufs=1) as cpool:
            # Resident weights / constants
            wk_sb = cpool.tile([P, 4, 2 * FG], F16)
            wv_sb = cpool.tile([P, 4, 2 * FG], F16)
            wq1_sb = cpool.tile([P, 4, FG], F16)
            wq2_sb = cpool.tile([P, 4, FG], F16)
            wo_sb = cpool.tile([P, 2, D], F16)
            bk_sb = cpool.tile([P, 2 * FG], F16)
            bv_sb = cpool.tile([P, 2 * FG], F16)
            bq1_sb = cpool.tile([P, 2], F32)
            bq2_sb = cpool.tile([P, 2], F32)
            e0_sb = cpool.tile([P, P], F16)
            sel_sb = cpool.tile([P, P], F32R)
            nc.sync.dma_start(e0_sb[:], e0[:])
            nc.sync.dma_start(bk_sb[:], bk12p[:])
            nc.sync.dma_start(bv_sb[:], bv12p[:])

            # Per-head-pair numerator/denominator lhsT built at phase boundary
            bd_sb = cpool.tile([P, 2, P], F16)      # blockdiag kv per pair
            dk_sb = cpool.tile([P, 2, 2], F16)      # ksum columns per pair

            # reciprocal staging (double-buffered); denominators land in rows
            # 0:2 (pair0) and 32:34 (pair1); other rows stay 1.0 so the
            # sel-matmul reads defined data everywhere it is nonzero.
            rcp_sb = cpool.tile([P, 2, 2, CHUNK], F32R)

            for _rep in range(repeats):
              # ---------------- Phase 1: k/v projections + state ----------------
              ctx_iop2 = tc.tile_pool(name="p2_io", bufs=3)
              iop2 = ctx_iop2.__enter__()
              with tc.tile_pool(name="p1_io", bufs=3) as iop, \
                   tc.tile_pool(name="p1_sil", bufs=4) as silp, \
                   tc.tile_pool(name="p1_kp", bufs=4 * GRP1 + 2) as kpp, \
                   tc.tile_pool(name="p1_ex", bufs=4) as exp_, \
                   tc.tile_pool(name="p1_ps", bufs=3, space="PSUM") as psp, \
                   tc.tile_pool(name="p1_st", bufs=1, space="PSUM") as stp:

                  state_ps = stp.tile([P, 2 * STW], F32)  # [kv|ksum2] per pair

                  # The backend scheduler freely interleaves independent
                  # scalar-engine instructions, which round-robins Silu and
                  # Exp and forces an act-table reload (~1.3us) per switch.
                  # gate_*: [128,1] tiles whose writes depend on the last act
                  # of the previous run; used as fake bias/scale operands to
                  # pin the Silu-run / Exp-run grouping via data deps.
                  gate = None         # zeros tile gating the next Silu run
                  last_exk = None
                  for g in range(NCHUNK // GRP1):
                      kprojs, vprojs, mnks = [], [], []
                      if last_exk is not None:
                          gate = exp_.tile([P, 1], F32, tag="gate")
                          nc.vector.tensor_scalar_mul(gate[:], last_exk[:, 0:1], 0.0)
                      for ci in range(GRP1):
                          c = g * GRP1 + ci
                          kT_c = iop.tile([P, 4, CHUNK], F16, tag="kT")
                          vT_c = iop.tile([P, 4, CHUNK], F16, tag="vT")
                          nc.sync.dma_start(kT_c[:], xk_t[:, c])
                          if c == 0:
                              for ki in range(4):
                                  nc.sync.dma_start(wk_sb[:, ki, :], wk12T_r[:, ki, :])
                          nc.sync.dma_start(vT_c[:], xv_t[:, c])
                          if c == 0:
                              for ki in range(4):
                                  nc.sync.dma_start(wv_sb[:, ki, :], wv12T_r[:, ki, :])
                          for s in range(SUBT):
                              tok = ds(s * P, P)
                              psk = psp.tile([P, 2 * FG], F32, tag="proj")
                              psv = psp.tile([P, 2 * FG], F32, tag="proj")
                              nc.tensor.matmul(psk[:], e0_sb[:], bk_sb[:], start=True, stop=False)
                              for ki in range(4):
                                  nc.tensor.matmul(psk[:], kT_c[:, ki, tok], wk_sb[:, ki, :],
                                                   start=False, stop=(ki == 3))
                              nc.tensor.matmul(psv[:], e0_sb[:], bv_sb[:], start=True, stop=False)
                              for ki in range(4):
                                  nc.tensor.matmul(psv[:], vT_c[:, ki, tok], wv_sb[:, ki, :],
                                                   start=False, stop=(ki == 3))
                              # silu halves (Silu-run on scalar engine)
                              gb = 0.0 if gate is None else gate[:, 0:1]
                              silk = silp.tile([P, FG], F32, tag="silk")
                              nc.scalar.activation(silk[:], psk[:, :FG], AF.Silu, bias=gb)
                              silv = silp.tile([P, FG], F32, tag="silv")
                              nc.scalar.activation(silv[:], psv[:, :FG], AF.Silu, bias=gb)
                              kproj = kpp.tile([P, FG], F32, tag="kproj")
                              nc.vector.tensor_tensor(kproj[:], psk[:, FG:], silk[:], OP.mult)
                              vproj = kpp.tile([P, 2, STW], F16, tag="vproj")
                              with nc.allow_low_precision(reason='fp16 matmul feed'):
                                  nc.vector.tensor_tensor(
                                      vproj[:, :, 0:P],
                                      psv[:, FG:].rearrange("p (g m) -> p g m", m=P),
                                      silv[:].rearrange("p (g m) -> p g m", m=P),
                                      OP.mult)
                              nc.gpsimd.memset(vproj[:, :, P:STW], 1.0)
                              mnk = kpp.tile([P, FG], F32, tag="mnk")
                              nc.vector.tensor_scalar_min(mnk[:], kproj[:], 0.0)
                              kprojs.append(kproj)
                              vprojs.append(vproj)
                              mnks.append(mnk)
                              last_silv = silv
                      # group tail: Exp-run + phi + state accumulation
                      egate = exp_.tile([P, 1], F32, tag="gate")
                      nc.vector.tensor_scalar_mul(egate[:], last_silv[:, 0:1], 0.0)
                      for i in range(4 * GRP1):
                          gi = g * 4 * GRP1 + i
                          first = (gi == 0)
                          last = (gi == S // P - 1)
                          exk = exp_.tile([P, FG], F32, tag="exk")
                          nc.scalar.activation(exk[:], mnks[i][:], AF.Exp,
                                               bias=egate[:, 0:1])
                          last_exk = exk
                          phik = exp_.tile([P, FG], F16, tag="phik")
                          with nc.allow_low_precision(reason='fp16 matmul feed'):
                              nc.vector.scalar_tensor_tensor(
                                  phik[:], kprojs[i][:], 1.0, exk[:], OP.add, OP.max)
                          # State: one matmul per pair covers kv (128 cols) and
                          # ksum (2 constant-1.0 cols of vproj). ONE PSUM bank
                          # holds both pairs' regions: start=True clears
                          # has_written for the WHOLE bank, so only the very
                          # first state matmul may use it; the other region's
                          # first matmul overwrites (bits cleared) and sets its
                          # own bits, after which everything accumulates.
                          for p in range(2):
                              nc.tensor.matmul(state_ps[:, ds(p * STW, STW)],
                                               phik[:, ts(p, P)], vprojs[i][:, p, :],
                                               start=(first and p == 0), stop=last,
                                               skip_group_check=True)

                  # --- phase boundary: build bd (blockdiag kv) and dk (ksum cols)
                  nc.sync.dma_start(bd_sb[:], bdz.rearrange("p (g m) -> p g m", m=P))
                  nc.sync.dma_start(dk_sb[:], dkz.rearrange("p (g m) -> p g m", m=2))
                  with nc.allow_low_precision(reason='fp16 state for PE matmul'):
                      for p in range(2):
                          nc.vector.tensor_copy(bd_sb[0:64, p, 0:64],
                                                state_ps[0:64, ds(p * STW, 64)])
                          nc.vector.tensor_copy(bd_sb[64:P, p, 64:P],
                                                state_ps[64:P, ds(p * STW + 64, 64)])
                          nc.vector.tensor_copy(dk_sb[0:64, p, 0:1],
                                                state_ps[0:64, ds(p * STW + P, 1)])
                          nc.vector.tensor_copy(dk_sb[64:P, p, 1:2],
                                                state_ps[64:P, ds(p * STW + P + 1, 1)])

              # phase-2 weights load late so phase-1's first tiles win the DMA queue
              nc.sync.dma_start(wq1_sb[:], wq1T_r[:])
              nc.sync.dma_start(wq2_sb[:], wq2T_r[:])
              nc.sync.dma_start(wo_sb[:], woT_r[:])
              nc.sync.dma_start(bq1_sb[:], bq1[:])
              nc.sync.dma_start(bq2_sb[:], bq2[:])
              nc.sync.dma_start(sel_sb[:], sel[:])
              nc.sync.dma_start(rcp_sb[:], rcp_init.rearrange("p (a b t) -> p a b t", a=2, b=2))

              # ---------------- Phase 2: q projections + attention + out -------
              with tc.tile_pool(name="p2_sb", bufs=4) as sbp2, \
                   tc.tile_pool(name="p2_qp", bufs=2 * GRP2 + 2) as qpp, \
                   tc.tile_pool(name="p2_ps", bufs=3, space="PSUM") as psp2, \
                   tc.tile_pool(name="p2_ps_big", bufs=3, space="PSUM") as psb2, \
                   tc.tile_pool(name="p2_ps_dn", bufs=2, space="PSUM") as psd2:

                  last_exq = None
                  for g in range(NCHUNK // GRP2):
                      saved = []
                      sgate = None
                      if last_exq is not None:
                          sgate = sbp2.tile([P, 1], F32, tag="gate2")
                          nc.vector.tensor_scalar(sgate[:], last_exq[:, 0:1],
                                                  0.0, 1.0, OP.mult, OP.add)
                      for ci in range(GRP2):
                          c = g * GRP2 + ci
                          qT_c = iop2.tile([P, 4, CHUNK], F16, tag="qT")
                          nc.sync.dma_start(qT_c[:], xq_t[:, c])
                          qps, mnqs = [], []
                          for m in range(2):
                              ps1 = psp2.tile([P, CHUNK], F32, tag="qproj")
                              ps2 = psp2.tile([P, CHUNK], F32, tag="qproj")
                              for ki in range(4):
                                  nc.tensor.matmul(ps1[:], wq1_sb[:, ki, ts(m, P)],
                                                   qT_c[:, ki, :], start=(ki == 0), stop=(ki == 3))
                              for ki in range(4):
                                  nc.tensor.matmul(ps2[:], wq2_sb[:, ki, ts(m, P)],
                                                   qT_c[:, ki, :], start=(ki == 0), stop=(ki == 3))
                              sil = sbp2.tile([P, CHUNK], F32, tag="sil")
                              nc.scalar.activation(sil[:], ps1[:], AF.Silu,
                                                   bias=bq1_sb[:, ds(m, 1)],
                                                   scale=(1.0 if sgate is None
                                                          else sgate[:, 0:1]))
                              qp = qpp.tile([P, CHUNK], F32, tag="qp")
                              nc.vector.scalar_tensor_tensor(
                                  qp[:], ps2[:], bq2_sb[:, ds(m, 1)], sil[:], OP.add, OP.mult)
                              mnq = qpp.tile([P, CHUNK], F32, tag="mnq")
                              nc.vector.tensor_scalar_min(mnq[:], qp[:], 0.0)
                              qps.append(qp)
                              mnqs.append(mnq)
                              last_sil = sil
                          saved.append((c, qps, mnqs))
                      # group tail: Exp-run + attention + output
                      egate2 = sbp2.tile([P, 1], F32, tag="gate2")
                      nc.vector.tensor_scalar_mul(egate2[:], last_sil[:, 0:1], 0.0)
                      for (c, qps, mnqs) in saved:
                          phiq = []
                          for m in range(2):
                              exq = sbp2.tile([P, CHUNK], F32, tag="exq")
                              nc.scalar.activation(exq[:], mnqs[m][:], AF.Exp,
                                                   bias=egate2[:, 0:1])
                              last_exq = exq
                              phm = sbp2.tile([P, CHUNK], F16, tag="phiq")
                              with nc.allow_low_precision(reason='fp16 matmul feed'):
                                  nc.vector.scalar_tensor_tensor(
                                      phm[:], qps[m][:], 1.0, exq[:], OP.add, OP.max)
                              phiq.append(phm)

                          # denominators: [2,CHUNK] per pair (MM dst must start
                          # at partition 0, so one PSUM tile per pair)
                          dn0 = psd2.tile([2, CHUNK], F32, tag="dn")
                          dn1 = psd2.tile([2, CHUNK], F32, tag="dn")
                          nc.tensor.matmul(dn0[:], dk_sb[:, 0, :], phiq[0][:],
                                           start=True, stop=True)
                          nc.tensor.matmul(dn1[:], dk_sb[:, 1, :], phiq[1][:],
                                           start=True, stop=True)
                          par = c % 2
                          # reciprocal_approx_fast (fp32-only) into a scratch,
                          # then a scalar Copy (table-neutral) rounds to f32r
                          # for the PE broadcast matmul.
                          for pp, dnp in ((0, dn0), (1, dn1)):
                              rsc = sbp2.tile([2, CHUNK], F32, tag="rsc")
                              nc.vector.reciprocal_approx_fast(rsc[:], dnp[:])
                              with nc.allow_low_precision(reason='f32r PE feed'):
                                  nc.scalar.copy(rcp_sb[0:2, par, pp, :], rsc[:])

                          outp = []
                          for p in range(2):
                              # numerator
                              nump = psb2.tile([P, CHUNK], F32, tag="mm")
                              nc.tensor.matmul(nump[:], bd_sb[:, p, :], phiq[p][:],
                                               start=True, stop=True)
                              # reciprocal rows broadcast to the pair's 128 partitions
                              rb = psb2.tile([P, CHUNK], F32, tag="mm")
                              nc.tensor.matmul(rb[:], sel_sb[:],
                                               rcp_sb[:, par, p, :],
                                               start=True, stop=True)
                              rbs = sbp2.tile([P, CHUNK], F32, tag="rbs")
                              nc.scalar.copy(rbs[:], rb[:])
                              att = sbp2.tile([P, CHUNK], F16, tag="att")
                              with nc.allow_low_precision(reason='fp16 matmul feed'):
                                  nc.vector.tensor_tensor(att[:], nump[:], rbs[:], OP.mult)
                              outp.append(att)

                          for s in range(SUBT):
                              po = psb2.tile([P, D], F32, tag="mm")
                              nc.tensor.matmul(po[:], outp[0][:, ts(s, P)], wo_sb[:, 0, :],
                                               start=True, stop=False)
                              nc.tensor.matmul(po[:], outp[1][:, ts(s, P)], wo_sb[:, 1, :],
                                               start=False, stop=True)
                              ob = sbp2.tile([P, D], F32, tag="ob")
                              if s % 2 == 0:
                                  nc.scalar.copy(ob[:], po[:])
                              else:
                                  nc.vector.tensor_copy(ob[:], po[:])
                              nc.sync.dma_start(out_r[c * SUBT + s], ob[:])

              ctx_iop2.__exit__(None, None, None)

    from concourse.library_overlay import lower_extended_insts
    lower_extended_insts(nc)
    _split_waits(nc)
    return nc


_NC_CACHE = None


def _get_nc():
    global _NC_CACHE
    if _NC_CACHE is None:
        _NC_CACHE = build_nc()
    return _NC_CACHE


def _prep_in_maps(inputs):
    return _build_in_maps(
        inputs["query"], inputs["key"], inputs["value"],
        inputs["q_w1"], inputs["q_w2"], inputs["k_w1"], inputs["k_w2"],
        inputs["v_w1"], inputs["v_w2"], inputs["out_w"],
        inputs["q_b1"], inputs["q_b2"], inputs["k_b1"], inputs["k_b2"],
        inputs["v_b1"], inputs["v_b2"])


def _tile_x(xb):
    """[S, D] -> [128, NCHUNK, 4, CHUNK] fp16 per-core input layout."""
    a = np.asarray(xb, np.float32).T.astype(np.float16)   # [D, S]
    return np.ascontiguousarray(
        a.reshape(4, P, NCHUNK, CHUNK).transpose(1, 2, 0, 3))


def _build_in_maps(query, key, value,
                   q_w1, q_w2, k_w1, k_w2, v_w1, v_w2, out_w,
                   q_b1, q_b2, k_b1, k_b2, v_b1, v_b2):
    e0 = np.zeros((P, P), np.float16); e0[0, :] = 1.0
    # sel[k, m]: reciprocal row j (j=0,1) -> partitions 64j..64j+63
    sel = np.zeros((P, P), np.float32)
    sel[0, 0:64] = 1.0
    sel[1, 64:128] = 1.0
    rcp_init = np.ones((P, 4 * CHUNK), np.float32)
    bdz = np.zeros((P, 2 * P), np.float16)
    dkz = np.zeros((P, 4), np.float16)

    xk = [_tile_x(np.asarray(key)[b]) for b in range(B)]
    xv = [_tile_x(np.asarray(value)[b]) for b in range(B)]
    xq = [_tile_x(np.asarray(query)[b]) for b in range(B)]

    in_maps = []
    for c in range(8):
        b, g = c // 2, c % 2
        Fs = slice(FG * g, FG * (g + 1))
        bk12p = np.zeros((P, 2 * FG), np.float16)
        bk12p[0] = np.concatenate([np.asarray(k_b1)[Fs], np.asarray(k_b2)[Fs]])
        bv12p = np.zeros((P, 2 * FG), np.float16)
        bv12p[0] = np.concatenate([np.asarray(v_b1)[Fs], np.asarray(v_b2)[Fs]])
        in_maps.append({
            "xk_t": xk[b],
            "xv_t": xv[b],
            "xq_t": xq[b],
            "wk12T": np.ascontiguousarray(np.concatenate(
                [np.asarray(k_w1)[Fs].T, np.asarray(k_w2)[Fs].T], axis=1)).astype(np.float16),
            "wv12T": np.ascontiguousarray(np.concatenate(
                [np.asarray(v_w1)[Fs].T, np.asarray(v_w2)[Fs].T], axis=1)).astype(np.float16),
            "wq1T": np.ascontiguousarray(np.asarray(q_w1)[Fs].T).astype(np.float16),
            "wq2T": np.ascontiguousarray(np.asarray(q_w2)[Fs].T).astype(np.float16),
            "bk12p": bk12p,
            "bv12p": bv12p,
            "bq1": np.ascontiguousarray(np.asarray(q_b1)[Fs].reshape(2, P).T.astype(np.float32)),
            "bq2": np.ascontiguousarray(np.asarray(q_b2)[Fs].reshape(2, P).T.astype(np.float32)),
            "woT": np.ascontiguousarray(np.asarray(out_w)[:, Fs].T).astype(np.float16),
            "e0": e0, "sel": sel, "bdz": bdz, "dkz": dkz,
            "rcp_init": rcp_init,
        })
    return in_maps


def kernel(query, key, value,
           q_w1, q_w2, k_w1, k_w2, v_w1, v_w2, out_w,
           q_b1, q_b2, k_b1, k_b2, v_b1, v_b2, out_b):
    in_maps = _build_in_maps(query, key, value,
                             q_w1, q_w2, k_w1, k_w2, v_w1, v_w2, out_w,
                             q_b1, q_b2, k_b1, k_b2, v_b1, v_b2)
    nc = _get_nc()
    res = run_bass_kernel_spmd(nc, in_maps, core_ids=list(range(8)))
    ob = np.asarray(out_b, dtype=np.float32)
    out = np.empty((B, S, D), np.float32)
    for b in range(B):
        out[b] = res.results[2 * b]["out"] + res.results[2 * b + 1]["out"] + ob
    return out
